# revision 11
# baseline (speedup 1.0000x reference)
"""Trainium2 Bass kernel for nn_InteractiveAttn (sparse_attention).

Reference computation:
    m = W_pq @ pool + b_pq                  # (H,)
    gated = encoder_outputs * m[None, :]    # (S, H)
    e = gated @ W_qh.T + b_qh               # (S, H)
    energies = e @ hidden                   # (S,)
    out = softmax(energies)[None, None, :]  # (1, 1, S)

Algebraic collapse: energies = enc @ ((m) * (W_qh.T @ hidden)) + b_qh.hidden.
The constant b_qh.hidden cancels in softmax, so
    out = softmax(enc @ (m * v)),  v = W_qh.T @ hidden.
This removes the (S,H)x(H,H) matmul entirely; the kernel is memory-bound
matvecs over enc (64MB) + the two weight matrices (32MB).

Sharding: hidden dim split 8 ways. Core r owns hidden slice H_r (256 cols):
reads enc[:, H_r] (transposed, 8MB), W_pq[H_r, :] (2MB), W_qh[:, H_r] (2MB),
computes w_r = (W_pq[H_r]@pool + b_pq[H_r]) * (W_qh[:,H_r].T@hidden) and
partial energies enc[:, H_r] @ w_r. One 32KB AllReduce combines partials;
every core then computes the full 8192-way softmax locally.
"""

import numpy as np

import concourse.bacc as bacc
import concourse.mybir as mybir
import concourse.tile as tile
from concourse import bass_isa, bass_utils

S = 8192
H = 2048
N_CORES = 8
HS = H // N_CORES  # 256 hidden columns per core
F32 = mybir.dt.float32

_nc_cache = {}


def build_nc(reps=1, with_collective=True, with_softmax=True,
             with_compute=True, split_coll=False):
    """Build (once) the SPMD Bass program run identically on all 8 cores.

    reps>1 emits the whole body (loads, compute, collective, softmax) that
    many times in straight line; the gating flags drop phases. Both are used
    only for slope-based benchmarking — the graded kernel uses defaults.
    """
    key = (reps, with_collective, with_softmax, with_compute, split_coll)
    if key in _nc_cache:
        return _nc_cache[key]

    nc = bacc.Bacc(
        "TRN2",
        target_bir_lowering=False,
        debug=False,
        enable_asserts=False,
        num_devices=N_CORES,
    )

    enc_t = nc.dram_tensor("enc_t", [HS, S], F32, kind="ExternalInput")
    wpq_t = nc.dram_tensor("wpq_t", [H, HS], F32, kind="ExternalInput")
    wqh = nc.dram_tensor("wqh", [H, HS], F32, kind="ExternalInput")
    pool = nc.dram_tensor("pool", [H], F32, kind="ExternalInput")
    hid = nc.dram_tensor("hid", [H], F32, kind="ExternalInput")
    bpq = nc.dram_tensor("bpq", [HS], F32, kind="ExternalInput")
    out_sm = nc.dram_tensor("out_sm", [S], F32, kind="ExternalOutput")

    KJ = H // 128  # 16 k-chunks of 128
    NB = HS // 128  # 2 output sub-blocks of the 256-col slice
    CH = 2048  # enc columns per DMA chunk
    NCH = S // CH  # 4 chunks per k-block

    def emit_body(tc, sb, ps, dr):
        inb = dr.tile([S], F32, tag="inb")

        if with_compute:
            # ---- small vectors: [p, j] = vec[j*128 + p]
            pool_sb = sb.tile([128, KJ], F32, tag="pool_sb")
            nc.sync.dma_start(pool_sb[:], pool.ap().rearrange("(j p) -> p j", p=128))
            hid_sb = sb.tile([128, KJ], F32, tag="hid_sb")
            nc.sync.dma_start(hid_sb[:], hid.ap().rearrange("(j p) -> p j", p=128))
            bpq_sb = sb.tile([128, NB], F32, tag="bpq_sb")
            nc.sync.dma_start(bpq_sb[:], bpq.ap().rearrange("(b p) -> p b", p=128))

            # ---- weights, k-major: [p, j*HS + i] = Wx[j*128 + p, i]
            wpq_sb = sb.tile([128, KJ * HS], F32, tag="wpq_sb")
            nc.sync.dma_start(
                wpq_sb[:].rearrange("p (j i) -> p j i", j=KJ),
                wpq_t.ap().rearrange("(j p) i -> p j i", p=128),
            )
            wqh_sb = sb.tile([128, KJ * HS], F32, tag="wqh_sb")
            nc.sync.dma_start(
                wqh_sb[:].rearrange("p (j i) -> p j i", j=KJ),
                wqh.ap().rearrange("(j p) i -> p j i", p=128),
            )

            # ---- m, v matvecs -> mv_ps cols: [m0 m1 v0 v1]
            mv_ps = ps.tile([128, 2 * NB], F32, tag="mv_ps")
            for t, vec_sb in ((0, pool_sb), (1, hid_sb)):
                w_mat = wpq_sb if t == 0 else wqh_sb
                for blk in range(NB):
                    col = t * NB + blk
                    for j in range(KJ):
                        nc.tensor.matmul(
                            mv_ps[:, col:col + 1],
                            w_mat[:, j * HS + blk * 128: j * HS + (blk + 1) * 128],
                            vec_sb[:, j:j + 1],
                            start=(j == 0),
                            stop=(j == KJ - 1),
                        )

            # ---- w = (m + b_pq) * v; w_sb[p, b] = w[b*128 + p]
            w_sb = sb.tile([128, NB], F32, tag="w_sb")
            nc.vector.tensor_add(w_sb[:], mv_ps[:, 0:NB], bpq_sb[:])
            nc.vector.tensor_mul(w_sb[:], w_sb[:], mv_ps[:, NB:2 * NB])

            # ---- stream enc, partial energies -> DRAM bounce for AllReduce
            for c in range(NCH):
                enc0 = sb.tile([128, CH], F32, tag="enc", bufs=4)
                nc.sync.dma_start(enc0[:], enc_t.ap()[0:128, c * CH:(c + 1) * CH])
                enc1 = sb.tile([128, CH], F32, tag="enc", bufs=4)
                nc.sync.dma_start(enc1[:], enc_t.ap()[128:256, c * CH:(c + 1) * CH])
                for q in range(CH // 512):
                    nn = c * (CH // 512) + q
                    e_ps = ps.tile([1, 512], F32, tag="eps", bufs=4)
                    nc.tensor.matmul(
                        e_ps[:], w_sb[:, 0:1], enc0[:, q * 512:(q + 1) * 512],
                        start=True, stop=False)
                    nc.tensor.matmul(
                        e_ps[:], w_sb[:, 1:2], enc1[:, q * 512:(q + 1) * 512],
                        start=False, stop=True)
                    e_sb = sb.tile([1, 512], F32, tag="edrain", bufs=4)
                    nc.scalar.copy(e_sb[:], e_ps[:])
                    nc.sync.dma_start(inb[:][nn * 512:(nn + 1) * 512], e_sb[:])

        if with_collective and split_coll:
            # Two half-size AllGathers: the first overlaps the second half
            # of the enc-energies compute.
            HSZ = S // 2
            acc = sb.tile([128, N_CORES * (S // 128)], F32, tag="acc")
            FP = S // 128  # 64
            HP = FP // 2   # free cols per half
            for h in range(2):
                gath_h = dr.tile([N_CORES * HSZ], F32, tag=f"gath{h}")
                nc.gpsimd.collective_compute(
                    "AllGather",
                    mybir.AluOpType.bypass,
                    replica_groups=[list(range(N_CORES))],
                    ins=[inb[:][h * HSZ:(h + 1) * HSZ].opt()],
                    outs=[gath_h.opt()],
                )
                for r in range(N_CORES):
                    nc.sync.dma_start(
                        acc[h * 64:(h + 1) * 64, r * FP:(r + 1) * FP],
                        gath_h[:][r * HSZ:(r + 1) * HSZ]
                        .rearrange("(p f) -> p f", p=64))
        elif with_collective:
            # AllGather the 8 partial-energy vectors (~5x cheaper than an
            # 8-rank AllReduce here), then each core reduces locally on DVE.
            gath = dr.tile([N_CORES * S], F32, tag="gath")
            nc.gpsimd.collective_compute(
                "AllGather",
                mybir.AluOpType.bypass,
                replica_groups=[list(range(N_CORES))],
                ins=[inb.opt()],
                outs=[gath.opt()],
            )
            # acc[p, r*64+f] = partial_r[p*64 + f]
            acc = sb.tile([128, N_CORES * (S // 128)], F32, tag="acc")
            FP = S // 128  # 64
            for r in range(N_CORES):
                nc.sync.dma_start(
                    acc[:, r * FP:(r + 1) * FP],
                    gath[:][r * S:(r + 1) * S].rearrange("(p f) -> p f", p=128))
        else:
            acc = None

        if with_softmax:
            # ---- softmax over the full 8192 energies: [p, f] = e[p*64 + f]
            esb = sb.tile([128, S // 128], F32, tag="esb")
            if acc is not None:
                # view acc as (p, f, r) [r strided] and sum over r
                FP = S // 128
                acc_v = acc[:].rearrange("p (r f) -> p f r", r=N_CORES)
                nc.vector.tensor_reduce(
                    esb[:], acc_v, axis=mybir.AxisListType.X,
                    op=mybir.AluOpType.add)
            else:
                nc.sync.dma_start(
                    esb[:], inb[:].rearrange("(p f) -> p f", p=128))
            rmax = sb.tile([128, 1], F32, tag="rmax")
            nc.vector.reduce_max(rmax[:], esb[:], axis=mybir.AxisListType.X)
            gmax = sb.tile([128, 1], F32, tag="gmax")
            nc.gpsimd.partition_all_reduce(
                gmax[:], rmax[:], channels=128, reduce_op=bass_isa.ReduceOp.max)
            ngmax = sb.tile([128, 1], F32, tag="ngmax")
            nc.scalar.mul(ngmax[:], gmax[:], -1.0)
            prob = sb.tile([128, S // 128], F32, tag="prob")
            rsum = sb.tile([128, 1], F32, tag="rsum")
            nc.scalar.activation(
                prob[:], esb[:], mybir.ActivationFunctionType.Exp,
                bias=ngmax[:], scale=1.0, accum_out=rsum[:])
            gsum = sb.tile([128, 1], F32, tag="gsum")
            nc.gpsimd.partition_all_reduce(
                gsum[:], rsum[:], channels=128, reduce_op=bass_isa.ReduceOp.add)
            rinv = sb.tile([128, 1], F32, tag="rinv")
            nc.vector.reciprocal(rinv[:], gsum[:])
            nc.vector.tensor_scalar_mul(prob[:], prob[:], rinv[:])
            nc.sync.dma_start(
                out_sm.ap().rearrange("(p f) -> p f", p=128), prob[:])
        else:
            nc.sync.dma_start(out_sm.ap(), inb[:])

    with tile.TileContext(nc) as tc:
        with tc.tile_pool(name="sb", bufs=1) as sb, \
             tc.tile_pool(name="ps", bufs=1, space="PSUM") as ps, \
             tc.tile_pool(name="dr", bufs=1, space="DRAM") as dr:
            for _rep in range(reps):
                if _rep:
                    tc.strict_bb_all_engine_barrier()
                emit_body(tc, sb, ps, dr)

    nc.finalize()
    _nc_cache[key] = nc
    return nc


def make_in_maps(hidden, encoder_outputs, pool_output, W_pq, b_pq, W_qh):
    hidden = np.asarray(hidden, dtype=np.float32)
    encoder_outputs = np.asarray(encoder_outputs, dtype=np.float32)
    pool_output = np.asarray(pool_output, dtype=np.float32)
    W_pq = np.asarray(W_pq, dtype=np.float32)
    b_pq = np.asarray(b_pq, dtype=np.float32)
    W_qh = np.asarray(W_qh, dtype=np.float32)

    enc_t_full = np.ascontiguousarray(encoder_outputs.T)  # (H, S)
    pool_v = np.ascontiguousarray(pool_output.reshape(-1))
    hid_v = np.ascontiguousarray(hidden.reshape(-1))
    in_maps = []
    for r in range(N_CORES):
        sl = slice(r * HS, (r + 1) * HS)
        in_maps.append({
            "enc_t": np.ascontiguousarray(enc_t_full[sl]),      # (HS, S)
            "wpq_t": np.ascontiguousarray(W_pq[sl].T),          # (H, HS)
            "wqh": np.ascontiguousarray(W_qh[:, sl]),           # (H, HS)
            "pool": pool_v,
            "hid": hid_v,
            "bpq": np.ascontiguousarray(b_pq[sl]),
        })
    return in_maps


def kernel(hidden, encoder_outputs, pool_output, W_pq, b_pq, W_qh, b_qh=None,
           **_unused):
    # b_qh adds a constant to every energy; softmax is shift-invariant, so it
    # is not an input to the device program.
    nc = build_nc()
    in_maps = make_in_maps(hidden, encoder_outputs, pool_output, W_pq, b_pq, W_qh)
    res = bass_utils.run_bass_kernel_spmd(
        nc, in_maps, core_ids=list(range(N_CORES)), trace=False)
    return res.results[0]["out_sm"].reshape(1, 1, S)


# revision 12
# speedup vs baseline: 6.0384x; 6.0384x over previous
"""Trainium2 Bass kernel for nn_InteractiveAttn (sparse_attention).

Reference computation:
    m = W_pq @ pool + b_pq                  # (H,)
    gated = encoder_outputs * m[None, :]    # (S, H)
    e = gated @ W_qh.T + b_qh               # (S, H)
    energies = e @ hidden                   # (S,)
    out = softmax(energies)[None, None, :]  # (1, 1, S)

Algebraic collapse: energies = enc @ ((m) * (W_qh.T @ hidden)) + b_qh.hidden.
The constant b_qh.hidden cancels in softmax, so
    out = softmax(enc @ (m * v)),  v = W_qh.T @ hidden.
This removes the (S,H)x(H,H) matmul entirely; the kernel is memory-bound
matvecs over enc (64MB) + the two weight matrices (32MB).

Sharding: hidden dim split 8 ways. Core r owns hidden slice H_r (256 cols):
reads enc[:, H_r] (transposed, 8MB), W_pq[H_r, :] (2MB), W_qh[:, H_r] (2MB),
computes w_r = (W_pq[H_r]@pool + b_pq[H_r]) * (W_qh[:,H_r].T@hidden) and
partial energies enc[:, H_r] @ w_r. One 32KB AllReduce combines partials;
every core then computes the full 8192-way softmax locally.
"""

import numpy as np

import concourse.bacc as bacc
import concourse.mybir as mybir
import concourse.tile as tile
from concourse import bass_isa, bass_utils

S = 8192
H = 2048
N_CORES = 8
HS = H // N_CORES  # 256 hidden columns per core
F32 = mybir.dt.float32

_nc_cache = {}


def build_nc(reps=1, with_collective=True, with_softmax=True,
             with_compute=True, split_coll=False, fast_sm=False):
    """Build (once) the SPMD Bass program run identically on all 8 cores.

    reps>1 emits the whole body (loads, compute, collective, softmax) that
    many times in straight line; the gating flags drop phases. Both are used
    only for slope-based benchmarking — the graded kernel uses defaults.
    """
    key = (reps, with_collective, with_softmax, with_compute, split_coll, fast_sm)
    if key in _nc_cache:
        return _nc_cache[key]

    nc = bacc.Bacc(
        "TRN2",
        target_bir_lowering=False,
        debug=False,
        enable_asserts=False,
        num_devices=N_CORES,
    )

    enc_t = nc.dram_tensor("enc_t", [HS, S], F32, kind="ExternalInput")
    wpq_t = nc.dram_tensor("wpq_t", [H, HS], F32, kind="ExternalInput")
    wqh = nc.dram_tensor("wqh", [H, HS], F32, kind="ExternalInput")
    pool = nc.dram_tensor("pool", [H], F32, kind="ExternalInput")
    hid = nc.dram_tensor("hid", [H], F32, kind="ExternalInput")
    bpq = nc.dram_tensor("bpq", [HS], F32, kind="ExternalInput")
    out_sm = nc.dram_tensor("out_sm", [S], F32, kind="ExternalOutput")

    KJ = H // 128  # 16 k-chunks of 128
    NB = HS // 128  # 2 output sub-blocks of the 256-col slice
    CH = 2048  # enc columns per DMA chunk
    NCH = S // CH  # 4 chunks per k-block

    def emit_body(tc, sb, ps, dr):
        inb = dr.tile([S], F32, tag="inb")

        if with_compute:
            # ---- small vectors: [p, j] = vec[j*128 + p]
            pool_sb = sb.tile([128, KJ], F32, tag="pool_sb")
            nc.sync.dma_start(pool_sb[:], pool.ap().rearrange("(j p) -> p j", p=128))
            hid_sb = sb.tile([128, KJ], F32, tag="hid_sb")
            nc.sync.dma_start(hid_sb[:], hid.ap().rearrange("(j p) -> p j", p=128))
            bpq_sb = sb.tile([128, NB], F32, tag="bpq_sb")
            nc.sync.dma_start(bpq_sb[:], bpq.ap().rearrange("(b p) -> p b", p=128))

            # ---- weights, k-major: [p, j*HS + i] = Wx[j*128 + p, i]
            wpq_sb = sb.tile([128, KJ * HS], F32, tag="wpq_sb")
            nc.sync.dma_start(
                wpq_sb[:].rearrange("p (j i) -> p j i", j=KJ),
                wpq_t.ap().rearrange("(j p) i -> p j i", p=128),
            )
            wqh_sb = sb.tile([128, KJ * HS], F32, tag="wqh_sb")
            nc.sync.dma_start(
                wqh_sb[:].rearrange("p (j i) -> p j i", j=KJ),
                wqh.ap().rearrange("(j p) i -> p j i", p=128),
            )

            # ---- m, v matvecs -> mv_ps cols: [m0 m1 v0 v1]
            mv_ps = ps.tile([128, 2 * NB], F32, tag="mv_ps")
            for t, vec_sb in ((0, pool_sb), (1, hid_sb)):
                w_mat = wpq_sb if t == 0 else wqh_sb
                for blk in range(NB):
                    col = t * NB + blk
                    for j in range(KJ):
                        nc.tensor.matmul(
                            mv_ps[:, col:col + 1],
                            w_mat[:, j * HS + blk * 128: j * HS + (blk + 1) * 128],
                            vec_sb[:, j:j + 1],
                            start=(j == 0),
                            stop=(j == KJ - 1),
                        )

            # ---- w = (m + b_pq) * v; w_sb[p, b] = w[b*128 + p]
            w_sb = sb.tile([128, NB], F32, tag="w_sb")
            nc.vector.tensor_add(w_sb[:], mv_ps[:, 0:NB], bpq_sb[:])
            nc.vector.tensor_mul(w_sb[:], w_sb[:], mv_ps[:, NB:2 * NB])

            # ---- stream enc, partial energies -> DRAM bounce for AllReduce
            for c in range(NCH):
                enc0 = sb.tile([128, CH], F32, tag="enc", bufs=4)
                nc.sync.dma_start(enc0[:], enc_t.ap()[0:128, c * CH:(c + 1) * CH])
                enc1 = sb.tile([128, CH], F32, tag="enc", bufs=4)
                nc.sync.dma_start(enc1[:], enc_t.ap()[128:256, c * CH:(c + 1) * CH])
                for q in range(CH // 512):
                    nn = c * (CH // 512) + q
                    e_ps = ps.tile([1, 512], F32, tag="eps", bufs=4)
                    nc.tensor.matmul(
                        e_ps[:], w_sb[:, 0:1], enc0[:, q * 512:(q + 1) * 512],
                        start=True, stop=False)
                    nc.tensor.matmul(
                        e_ps[:], w_sb[:, 1:2], enc1[:, q * 512:(q + 1) * 512],
                        start=False, stop=True)
                    e_sb = sb.tile([1, 512], F32, tag="edrain", bufs=4)
                    nc.scalar.copy(e_sb[:], e_ps[:])
                    nc.sync.dma_start(inb[:][nn * 512:(nn + 1) * 512], e_sb[:])

        if with_collective and split_coll:
            # Two half-size AllGathers: the first overlaps the second half
            # of the enc-energies compute.
            HSZ = S // 2
            acc = sb.tile([128, N_CORES * (S // 128)], F32, tag="acc")
            FP = S // 128  # 64
            HP = FP // 2   # free cols per half
            for h in range(2):
                gath_h = dr.tile([N_CORES * HSZ], F32, tag=f"gath{h}")
                nc.gpsimd.collective_compute(
                    "AllGather",
                    mybir.AluOpType.bypass,
                    replica_groups=[list(range(N_CORES))],
                    ins=[inb[:][h * HSZ:(h + 1) * HSZ].opt()],
                    outs=[gath_h.opt()],
                )
                for r in range(N_CORES):
                    nc.sync.dma_start(
                        acc[h * 64:(h + 1) * 64, r * FP:(r + 1) * FP],
                        gath_h[:][r * HSZ:(r + 1) * HSZ]
                        .rearrange("(p f) -> p f", p=64))
        elif with_collective:
            # AllGather the 8 partial-energy vectors (~5x cheaper than an
            # 8-rank AllReduce here), then each core reduces locally on DVE.
            gath = dr.tile([N_CORES * S], F32, tag="gath")
            nc.gpsimd.collective_compute(
                "AllGather",
                mybir.AluOpType.bypass,
                replica_groups=[list(range(N_CORES))],
                ins=[inb.opt()],
                outs=[gath.opt()],
            )
            # acc[p, r*64+f] = partial_r[p*64 + f]
            acc = sb.tile([128, N_CORES * (S // 128)], F32, tag="acc")
            FP = S // 128  # 64
            for r in range(N_CORES):
                nc.sync.dma_start(
                    acc[:, r * FP:(r + 1) * FP],
                    gath[:][r * S:(r + 1) * S].rearrange("(p f) -> p f", p=128))
        else:
            acc = None

        if with_softmax:
            # ---- softmax over the full 8192 energies: [p, f] = e[p*64 + f]
            esb = sb.tile([128, S // 128], F32, tag="esb")
            if acc is not None:
                # view acc as (p, f, r) [r strided] and sum over r
                FP = S // 128
                acc_v = acc[:].rearrange("p (r f) -> p f r", r=N_CORES)
                nc.vector.tensor_reduce(
                    esb[:], acc_v, axis=mybir.AxisListType.X,
                    op=mybir.AluOpType.add)
            else:
                nc.sync.dma_start(
                    esb[:], inb[:].rearrange("(p f) -> p f", p=128))
            rmax = sb.tile([128, 1], F32, tag="rmax")
            nc.vector.reduce_max(rmax[:], esb[:], axis=mybir.AxisListType.X)
            prob = sb.tile([128, S // 128], F32, tag="prob")
            rsum = sb.tile([128, 1], F32, tag="rsum")
            if fast_sm:
                ones = sb.tile([1, 128], F32, tag="ones")
                nc.vector.memset(ones[:], 1.0)
                rowbuf = sb.tile([1, 128], F32, tag="rowbuf")
                nc.sync.dma_start(rowbuf[:], rmax[:, 0:1])
                ngrow = sb.tile([1, 1], F32, tag="ngrow")
                nc.vector.tensor_reduce(
                    ngrow[:], rowbuf[:], axis=mybir.AxisListType.X,
                    op=mybir.AluOpType.max, negate=True)
                bps = ps.tile([128, 1], F32, tag="bps")
                nc.tensor.matmul(bps[:], ones[:], ngrow[:], start=True, stop=True)
                ngmax = sb.tile([128, 1], F32, tag="ngmax")
                nc.scalar.copy(ngmax[:], bps[:])
            else:
                gmax = sb.tile([128, 1], F32, tag="gmax")
                nc.gpsimd.partition_all_reduce(
                    gmax[:], rmax[:], channels=128,
                    reduce_op=bass_isa.ReduceOp.max)
                ngmax = sb.tile([128, 1], F32, tag="ngmax")
                nc.scalar.mul(ngmax[:], gmax[:], -1.0)
            nc.scalar.activation(
                prob[:], esb[:], mybir.ActivationFunctionType.Exp,
                bias=ngmax[:], scale=1.0, accum_out=rsum[:])
            if fast_sm:
                rowbuf2 = sb.tile([1, 128], F32, tag="rowbuf2")
                nc.sync.dma_start(rowbuf2[:], rsum[:, 0:1])
                grow = sb.tile([1, 1], F32, tag="grow")
                nc.vector.tensor_reduce(
                    grow[:], rowbuf2[:], axis=mybir.AxisListType.X,
                    op=mybir.AluOpType.add)
                rrow = sb.tile([1, 1], F32, tag="rrow")
                nc.vector.reciprocal(rrow[:], grow[:])
                bps2 = ps.tile([128, 1], F32, tag="bps2")
                nc.tensor.matmul(bps2[:], ones[:], rrow[:], start=True, stop=True)
                rinv = sb.tile([128, 1], F32, tag="rinv")
                nc.scalar.copy(rinv[:], bps2[:])
            else:
                gsum = sb.tile([128, 1], F32, tag="gsum")
                nc.gpsimd.partition_all_reduce(
                    gsum[:], rsum[:], channels=128,
                    reduce_op=bass_isa.ReduceOp.add)
                rinv = sb.tile([128, 1], F32, tag="rinv")
                nc.vector.reciprocal(rinv[:], gsum[:])
            nc.vector.tensor_scalar_mul(prob[:], prob[:], rinv[:])
            nc.sync.dma_start(
                out_sm.ap().rearrange("(p f) -> p f", p=128), prob[:])
        else:
            nc.sync.dma_start(out_sm.ap(), inb[:])

    with tile.TileContext(nc) as tc:
        with tc.tile_pool(name="sb", bufs=1) as sb, \
             tc.tile_pool(name="ps", bufs=1, space="PSUM") as ps, \
             tc.tile_pool(name="dr", bufs=1, space="DRAM") as dr:
            for _rep in range(reps):
                if _rep:
                    tc.strict_bb_all_engine_barrier()
                emit_body(tc, sb, ps, dr)

    nc.finalize()
    _nc_cache[key] = nc
    return nc


def make_in_maps(hidden, encoder_outputs, pool_output, W_pq, b_pq, W_qh):
    hidden = np.asarray(hidden, dtype=np.float32)
    encoder_outputs = np.asarray(encoder_outputs, dtype=np.float32)
    pool_output = np.asarray(pool_output, dtype=np.float32)
    W_pq = np.asarray(W_pq, dtype=np.float32)
    b_pq = np.asarray(b_pq, dtype=np.float32)
    W_qh = np.asarray(W_qh, dtype=np.float32)

    enc_t_full = np.ascontiguousarray(encoder_outputs.T)  # (H, S)
    pool_v = np.ascontiguousarray(pool_output.reshape(-1))
    hid_v = np.ascontiguousarray(hidden.reshape(-1))
    in_maps = []
    for r in range(N_CORES):
        sl = slice(r * HS, (r + 1) * HS)
        in_maps.append({
            "enc_t": np.ascontiguousarray(enc_t_full[sl]),      # (HS, S)
            "wpq_t": np.ascontiguousarray(W_pq[sl].T),          # (H, HS)
            "wqh": np.ascontiguousarray(W_qh[:, sl]),           # (H, HS)
            "pool": pool_v,
            "hid": hid_v,
            "bpq": np.ascontiguousarray(b_pq[sl]),
        })
    return in_maps


def kernel(hidden, encoder_outputs, pool_output, W_pq, b_pq, W_qh, b_qh=None,
           **_unused):
    # b_qh adds a constant to every energy; softmax is shift-invariant, so it
    # is not an input to the device program.
    nc = build_nc()
    in_maps = make_in_maps(hidden, encoder_outputs, pool_output, W_pq, b_pq, W_qh)
    res = bass_utils.run_bass_kernel_spmd(
        nc, in_maps, core_ids=list(range(N_CORES)), trace=False)
    return res.results[0]["out_sm"].reshape(1, 1, S)
